# revision 34
# baseline (speedup 1.0000x reference)
"""Trainium2 Bass kernel for BiaffinePairing.

Computes S = (T @ W) @ A^T + T @ U[:H] + (A @ U[H:]).T + b  -> [4096, 4096] f32.

Strategy (8 NeuronCores, data-parallel over T's row dim n):
  - Host-side layout prep only (no math): transpose T and A so the
    contraction dim H=1024 lies on SBUF partitions; shard T^T's columns
    (the n dim) 8 ways; replicate A^T, W, and the U halves. Matmul inputs
    are pre-cast to fp16 on the host (the DMA streams halve; measured
    3.7e-4 relative error end-to-end vs the fp32 reference).
  - Per core: mm1 computes TWt[h_out, n] = (T_shard @ W)^T accumulating in
    PSUM; the rank-1 term 1_n (x) (A @ u_a)^T folds in by adding u_a[h] as
    a per-partition bias on mm1's PSUM->SBUF copy (since
    (TW + 1 (x) u_a^T) @ A^T = TW@A^T + 1 (x) (A@u_a)^T).
  - tvec[n] = T_shard @ u_t + b via tiny matmuls; added as the per-partition
    bias on mm2's PSUM->SBUF copies.
  - mm2 computes S_shard[n, m] = sum_k TWt[k]^T @ At[k] over m-chunks.

Schedule notes (why the structure looks the way it does):
  - Dummy warmup matmuls run during the ~7us framework preamble so the PE
    HAM clock-gate reaches 8/8 before real work.
  - mm1 is k-outer over 4 PSUM banks in two ho-half passes: each k step
    needs only W/tT k-tile k, so the PE chases the load DMAs.
  - Load DMAs alternate between the two HWDGE FIFOs (sync/scalar) to halve
    the ~650ns-per-dma_start issue serialization; out stores also go on
    scalar so they never head-of-line-block the at-chunk loads on sync.
"""

import numpy as np

import concourse.bacc as bacc
import concourse.mybir as mybir
from concourse.tile import TileContext
from concourse.bass_utils import run_bass_kernel_spmd

H = 1024          # hidden dim (contraction)
N_TOT = 4096      # rows of target_spans
M_TOT = 4096      # rows of argument_spans
N_CORES = 8
NSH = N_TOT // N_CORES   # 512 n rows per core
KT = H // 128            # 8 contraction k-tiles
NI = NSH // 128          # 4 n-tiles of 128 per core
MCH = 1024               # m-chunk width
MC = M_TOT // MCH        # 4 m-chunks
MH = MCH // 512          # 512-wide psum sub-slices per chunk

F32 = mybir.dt.float32
F16 = mybir.dt.float16

_NC_CACHE = {}


def _build(b_val: float):
    nc = bacc.Bacc("TRN2", target_bir_lowering=False, debug=False,
                   num_devices=N_CORES)

    tT = nc.dram_tensor("tT", [H, NSH], F16, kind="ExternalInput")
    aT = nc.dram_tensor("aT", [H, M_TOT], F16, kind="ExternalInput")
    W = nc.dram_tensor("W", [H, H], F16, kind="ExternalInput")
    # Natural-layout T shard + u_t replicated across 128 partitions: used
    # to compute tvec on the DVE (free-dim reduce) with zero PE time.
    tn = nc.dram_tensor("tn", [NSH, H], F16, kind="ExternalInput")
    utb = nc.dram_tensor("utb", [128, H], F16, kind="ExternalInput")
    ua = nc.dram_tensor("ua", [H, 1], F32, kind="ExternalInput")
    out = nc.dram_tensor("out", [NSH, M_TOT], F32, kind="ExternalOutput")

    # DRAM views with the k-tile index split out: row kt*128 + p.
    tT_v = tT.rearrange("(kt p) n -> p kt n", p=128)
    aT_v = aT.rearrange("(kt p) m -> p kt m", p=128)
    W_v = W.rearrange("(kt p) f -> p kt f", p=128)
    tn_v = tn.rearrange("(ni p) h -> p ni h", p=128)
    ua_v = ua.rearrange("(kt p) one -> p (kt one)", p=128)

    with TileContext(nc) as tc:
        with (
            tc.tile_pool(name="const", bufs=1) as cpool,
            tc.tile_pool(name="achunk", bufs=4) as apool,
            tc.tile_pool(name="outbuf", bufs=6) as opool,
            tc.tile_pool(name="ps1", bufs=1, space="PSUM") as ps1pool,
            tc.tile_pool(name="ps2", bufs=4, space="PSUM") as ps2pool,
        ):
            # ---- constant loads, per k-tile, alternating FIFOs ----
            w_sb = cpool.tile([128, KT, H], F16, tag="w")
            tT_sb = cpool.tile([128, KT, NSH], F16, tag="tT")
            ua_sb = cpool.tile([128, KT], F32, tag="ua")
            tn_sb = cpool.tile([128, NI, H], F16, tag="tn")
            utb_sb = cpool.tile([128, H], F16, tag="utb")
            nc.scalar.dma_start(out=ua_sb[:], in_=ua_v[:])
            for k in range(KT):
                eng_w = nc.sync if k % 2 == 0 else nc.scalar
                eng_t = nc.scalar if k % 2 == 0 else nc.sync
                eng_w.dma_start(out=w_sb[:, k, :], in_=W_v[:, k, :])
                eng_t.dma_start(out=tT_sb[:, k, :], in_=tT_v[:, k, :])
            # tvec inputs load after the mm1-critical tiles; they're not
            # needed until the first out-copy (~20us in).
            nc.sync.dma_start(out=utb_sb[:], in_=utb[:, :])
            nc.scalar.dma_start(out=tn_sb[:], in_=tn_v[:])

            # ---- PE warmup: the first ~7us are framework preamble + DMA
            # ramp with the PE idle, which leaves the HAM clock-gate at
            # K=4/8 (half clock) well into mm1. Dummy matmuls on zeroed
            # tiles trip the HAM busy-window during that dead time so mm1
            # runs at full clock. ----
            warm_w = cpool.tile([128, 128], F16, tag="warm_w")
            warm_in = cpool.tile([128, 512], F16, tag="warm_in")
            nc.vector.memset(warm_w[:], 0.0)
            nc.vector.memset(warm_in[:], 0.0)
            wps = ps1pool.tile([128, NSH], F32, tag="ps1_0", name="wps")
            for _ in range(16):
                nc.tensor.matmul(wps[:], warm_w[:], warm_in[:],
                                 start=True, stop=True)

            # ---- mm1: TWt[h_out, n] = (T @ W)^T, + u_a bias on copy-out.
            # Two ho-half passes, k-outer over 4 PSUM banks each: a k step
            # only needs the W/tT k-tile k, so the PE starts as soon as the
            # first pair of DMAs lands; pass B reuses the resident W. ----
            twt_sb = cpool.tile([128, KT, NSH], F16, tag="twt")
            ps1 = [ps1pool.tile([128, NSH], F32, tag=f"ps1_{j}",
                                name=f"ps1_{j}")
                   for j in range(4)]
            for half in range(2):
                for k in range(KT):
                    for j in range(4):
                        ho = half * 4 + j
                        nc.tensor.matmul(
                            ps1[j][:],
                            w_sb[:, k, ho * 128:(ho + 1) * 128],
                            tT_sb[:, k, :],
                            start=(k == 0),
                            stop=(k == KT - 1),
                        )
                for j in range(4):
                    ho = half * 4 + j
                    # TWt[ho] = psum + u_a[ho-tile] (per-partition bias),
                    # cast to fp16 for mm2. Copies land after the final
                    # k row; alternate DVE/ACT to halve the serial latency
                    # gating mm2's start.
                    if j % 2 == 0:
                        nc.vector.tensor_scalar_add(
                            out=twt_sb[:, ho, :], in0=ps1[j][:],
                            scalar1=ua_sb[:, ho:ho + 1],
                        )
                    else:
                        nc.scalar.activation(
                            out=twt_sb[:, ho, :], in_=ps1[j][:],
                            func=mybir.ActivationFunctionType.Identity,
                            bias=ua_sb[:, ho:ho + 1],
                        )


            # ---- tvec[n] = T @ u_t + b on the DVE: elementwise multiply of
            # the natural-layout T tile by the replicated u_t row, reduced
            # along the free (h) dim, with b as the reduction's initial
            # value. Zero PE time; the DVE is otherwise idle here. ----
            tvec_sb = cpool.tile([128, NI], F32, tag="tvec")
            ttscratch = cpool.tile([128, H], F32, tag="ttscratch")
            for ni in range(NI):
                nc.vector.tensor_tensor(
                    out=ttscratch[:],
                    in0=tn_sb[:, ni, :],
                    in1=utb_sb[:],
                    op=mybir.AluOpType.mult,
                )
                nc.vector.tensor_reduce(
                    out=tvec_sb[:, ni:ni + 1], in_=ttscratch[:],
                    axis=mybir.AxisListType.X, op=mybir.AluOpType.add,
                )
            nc.vector.tensor_scalar_add(
                out=tvec_sb[:], in0=tvec_sb[:], scalar1=float(b_val),
            )

            # ---- mm2: S[n, m] = sum_k TWt[k]^T @ At[k], + tvec bias ----
            for c in range(MC):
                at_sb = apool.tile([128, KT, MCH], F16, tag="at")
                nc.sync.dma_start(
                    out=at_sb[:],
                    in_=aT_v[:, :, c * MCH:(c + 1) * MCH],
                )
                for ni in range(NI):
                    for h in range(MH):
                        ps = ps2pool.tile([128, 512], F32, tag="ps", name="ps")
                        for k in range(KT):
                            nc.tensor.matmul(
                                ps[:],
                                twt_sb[:, k, ni * 128:(ni + 1) * 128],
                                at_sb[:, k, h * 512:(h + 1) * 512],
                                start=(k == 0),
                                stop=(k == KT - 1),
                            )
                        o_sb = opool.tile([128, 512], F32, tag="o")
                        nc.vector.tensor_scalar_add(
                            out=o_sb[:], in0=ps[:],
                            scalar1=tvec_sb[:, ni:ni + 1],
                        )
                        # Stores go on the scalar HWDGE FIFO so they never
                        # head-of-line-block the at-loads on the sync FIFO.
                        nc.scalar.dma_start(
                            out=out[ni * 128:(ni + 1) * 128,
                                    c * MCH + h * 512:c * MCH + (h + 1) * 512],
                            in_=o_sb[:],
                        )

    nc.compile()
    return nc


def _get_nc(b_val: float):
    key = float(b_val)
    if key not in _NC_CACHE:
        _NC_CACHE[key] = _build(key)
    return _NC_CACHE[key]


def make_in_maps(target_spans, argument_spans, W, U, b):
    """Host-side layout prep: shard/transpose/cast the full inputs into the
    per-core input maps. Returns (in_maps, b_val)."""
    target_spans = np.asarray(target_spans, dtype=np.float32)
    argument_spans = np.asarray(argument_spans, dtype=np.float32)
    W = np.ascontiguousarray(np.asarray(W, dtype=np.float16))
    U = np.asarray(U, dtype=np.float32).reshape(2 * H, 1)
    b_val = float(np.asarray(b).reshape(-1)[0])

    tT = np.ascontiguousarray(target_spans.T.astype(np.float16))  # [H, N_TOT]
    aT = np.ascontiguousarray(argument_spans.T.astype(np.float16))  # [H, M_TOT]
    t16 = target_spans.astype(np.float16)                 # [N_TOT, H]
    utb = np.ascontiguousarray(
        np.repeat(U[:H].reshape(1, H), 128, axis=0).astype(np.float16))
    ua = np.ascontiguousarray(U[H:])

    in_maps = [
        {
            "tT": np.ascontiguousarray(tT[:, i * NSH:(i + 1) * NSH]),
            "aT": aT,
            "W": W,
            "tn": np.ascontiguousarray(t16[i * NSH:(i + 1) * NSH]),
            "utb": utb,
            "ua": ua,
        }
        for i in range(N_CORES)
    ]
    return in_maps, b_val


def kernel(target_spans, argument_spans, W, U, b):
    in_maps, b_val = make_in_maps(target_spans, argument_spans, W, U, b)
    nc = _get_nc(b_val)
    res = run_bass_kernel_spmd(nc, in_maps, core_ids=list(range(N_CORES)))
    out = np.concatenate(
        [res.results[i]["out"] for i in range(N_CORES)], axis=0
    )
    return out.astype(np.float32, copy=False)


# revision 37
# speedup vs baseline: 1.1086x; 1.1086x over previous
"""Trainium2 Bass kernel for BiaffinePairing.

Computes S = (T @ W) @ A^T + T @ U[:H] + (A @ U[H:]).T + b  -> [4096, 4096] f32.

Strategy (8 NeuronCores, data-parallel over T's row dim n):
  - Host-side layout prep only (no math): transpose T and A so the
    contraction dim H=1024 lies on SBUF partitions; shard T^T's columns
    (the n dim) 8 ways; replicate A^T, W, and the U halves. Matmul inputs
    are pre-cast to fp16 on the host (the DMA streams halve; measured
    3.7e-4 relative error end-to-end vs the fp32 reference).
  - Per core: mm1 computes TWt[h_out, n] = (T_shard @ W)^T accumulating in
    PSUM; the rank-1 term 1_n (x) (A @ u_a)^T folds in by adding u_a[h] as
    a per-partition bias on mm1's PSUM->SBUF copy (since
    (TW + 1 (x) u_a^T) @ A^T = TW@A^T + 1 (x) (A@u_a)^T).
  - tvec[n] = T_shard @ u_t + b via tiny matmuls; added as the per-partition
    bias on mm2's PSUM->SBUF copies.
  - mm2 computes S_shard[n, m] = sum_k TWt[k]^T @ At[k] over m-chunks.

Schedule notes (why the structure looks the way it does):
  - Dummy warmup matmuls run during the ~7us framework preamble so the PE
    HAM clock-gate reaches 8/8 before real work.
  - mm1 is k-outer over 4 PSUM banks in two ho-half passes: each k step
    needs only W/tT k-tile k, so the PE chases the load DMAs.
  - Load DMAs alternate between the two HWDGE FIFOs (sync/scalar) to halve
    the ~650ns-per-dma_start issue serialization; out stores also go on
    scalar so they never head-of-line-block the at-chunk loads on sync.
"""

import numpy as np

import concourse.bacc as bacc
import concourse.mybir as mybir
from concourse.tile import TileContext
from concourse.bass_utils import run_bass_kernel_spmd

H = 1024          # hidden dim (contraction)
N_TOT = 4096      # rows of target_spans
M_TOT = 4096      # rows of argument_spans
N_CORES = 8
NSH = N_TOT // N_CORES   # 512 n rows per core
KT = H // 128            # 8 contraction k-tiles
NI = NSH // 128          # 4 n-tiles of 128 per core
MCH = 1024               # m-chunk width
MC = M_TOT // MCH        # 4 m-chunks
MH = MCH // 512          # 512-wide psum sub-slices per chunk

F32 = mybir.dt.float32
F16 = mybir.dt.float16

_NC_CACHE = {}


def _build(b_val: float):
    nc = bacc.Bacc("TRN2", target_bir_lowering=False, debug=False,
                   num_devices=N_CORES)

    tT = nc.dram_tensor("tT", [H, NSH], F16, kind="ExternalInput")
    aT = nc.dram_tensor("aT", [H, M_TOT], F16, kind="ExternalInput")
    W = nc.dram_tensor("W", [H, H], F16, kind="ExternalInput")
    # ut comes in as [H, 2] (two identical columns -> a 2-wide moving
    # operand; both result columns equal tvec).
    ut = nc.dram_tensor("ut", [H, 2], F16, kind="ExternalInput")
    ua = nc.dram_tensor("ua", [H, 1], F32, kind="ExternalInput")
    out = nc.dram_tensor("out", [NSH, M_TOT], F32, kind="ExternalOutput")

    # DRAM views with the k-tile index split out: row kt*128 + p.
    tT_v = tT.rearrange("(kt p) n -> p kt n", p=128)
    aT_v = aT.rearrange("(kt p) m -> p kt m", p=128)
    W_v = W.rearrange("(kt p) f -> p kt f", p=128)
    ut_v = ut.rearrange("(kt p) two -> p kt two", p=128)
    ua_v = ua.rearrange("(kt p) one -> p (kt one)", p=128)

    with TileContext(nc) as tc:
        with (
            tc.tile_pool(name="const", bufs=1) as cpool,
            tc.tile_pool(name="achunk", bufs=4) as apool,
            tc.tile_pool(name="outbuf", bufs=6) as opool,
            tc.tile_pool(name="ps1", bufs=1, space="PSUM") as ps1pool,
            tc.tile_pool(name="ps2", bufs=4, space="PSUM") as ps2pool,
        ):
            # ---- constant loads, per k-tile, alternating FIFOs ----
            w_sb = cpool.tile([128, KT, H], F16, tag="w")
            tT_sb = cpool.tile([128, KT, NSH], F16, tag="tT")
            ua_sb = cpool.tile([128, KT], F32, tag="ua")
            ut_sb = cpool.tile([128, KT, 2], F16, tag="ut")
            nc.sync.dma_start(out=ut_sb[:], in_=ut_v[:])
            nc.scalar.dma_start(out=ua_sb[:], in_=ua_v[:])
            for k in range(KT):
                eng_w = nc.sync if k % 2 == 0 else nc.scalar
                eng_t = nc.scalar if k % 2 == 0 else nc.sync
                eng_w.dma_start(out=w_sb[:, k, :], in_=W_v[:, k, :])
                eng_t.dma_start(out=tT_sb[:, k, :], in_=tT_v[:, k, :])

            # ---- PE warmup: the first ~7us are framework preamble + DMA
            # ramp with the PE idle, which leaves the HAM clock-gate at
            # K=4/8 (half clock) well into mm1. Dummy matmuls on zeroed
            # tiles trip the HAM busy-window during that dead time so mm1
            # runs at full clock. ----
            warm_w = cpool.tile([128, 128], F16, tag="warm_w")
            warm_in = cpool.tile([128, 512], F16, tag="warm_in")
            nc.vector.memset(warm_w[:], 0.0)
            nc.vector.memset(warm_in[:], 0.0)
            wps = ps1pool.tile([128, NSH], F32, tag="ps1_0", name="wps")
            for _ in range(16):
                nc.tensor.matmul(wps[:], warm_w[:], warm_in[:],
                                 start=True, stop=True)

            # ---- mm1: TWt[h_out, n] = (T @ W)^T, + u_a bias on copy-out.
            # Two ho-half passes, k-outer over 4 PSUM banks each: a k step
            # only needs the W/tT k-tile k, so the PE starts as soon as the
            # first pair of DMAs lands; pass B reuses the resident W. ----
            twt_sb = cpool.tile([128, KT, NSH], F16, tag="twt")
            ps1 = [ps1pool.tile([128, NSH], F32, tag=f"ps1_{j}",
                                name=f"ps1_{j}")
                   for j in range(4)]
            for half in range(2):
                for k in range(KT):
                    for j in range(4):
                        ho = half * 4 + j
                        nc.tensor.matmul(
                            ps1[j][:],
                            w_sb[:, k, ho * 128:(ho + 1) * 128],
                            tT_sb[:, k, :],
                            start=(k == 0),
                            stop=(k == KT - 1),
                        )
                for j in range(4):
                    ho = half * 4 + j
                    # TWt[ho] = psum + u_a[ho-tile] (per-partition bias),
                    # cast to fp16 for mm2. Copies land after the final
                    # k row; alternate DVE/ACT to halve the serial latency
                    # gating mm2's start.
                    if j % 2 == 0:
                        nc.vector.tensor_scalar_add(
                            out=twt_sb[:, ho, :], in0=ps1[j][:],
                            scalar1=ua_sb[:, ho:ho + 1],
                        )
                    else:
                        nc.scalar.activation(
                            out=twt_sb[:, ho, :], in_=ps1[j][:],
                            func=mybir.ActivationFunctionType.Identity,
                            bias=ua_sb[:, ho:ho + 1],
                        )


            # ---- tvec[n] = T @ u_t + b on the DVE: elementwise multiply of
            # the natural-layout T tile by the replicated u_t row, reduced
            # along the free (h) dim, with b as the reduction's initial
            # value. Zero PE time; the DVE is otherwise idle here. ----
            tvec_sb = cpool.tile([128, NI], F32, tag="tvec")
            for ni in range(NI):
                psv = ps2pool.tile([128, 2], F32, tag="ps", name="psv")
                for k in range(KT):
                    nc.tensor.matmul(
                        psv[:],
                        tT_sb[:, k, ni * 128:(ni + 1) * 128],
                        ut_sb[:, k, :],
                        start=(k == 0),
                        stop=(k == KT - 1),
                    )
                nc.scalar.activation(
                    out=tvec_sb[:, ni:ni + 1], in_=psv[:, 0:1],
                    func=mybir.ActivationFunctionType.Identity,
                    bias=float(b_val),
                )

            # ---- mm2: S[n, m] = sum_k TWt[k]^T @ At[k], + tvec bias ----
            for c in range(MC):
                at_sb = apool.tile([128, KT, MCH], F16, tag="at")
                nc.sync.dma_start(
                    out=at_sb[:],
                    in_=aT_v[:, :, c * MCH:(c + 1) * MCH],
                )
                for ni in range(NI):
                    for h in range(MH):
                        ps = ps2pool.tile([128, 512], F32, tag="ps", name="ps")
                        for k in range(KT):
                            nc.tensor.matmul(
                                ps[:],
                                twt_sb[:, k, ni * 128:(ni + 1) * 128],
                                at_sb[:, k, h * 512:(h + 1) * 512],
                                start=(k == 0),
                                stop=(k == KT - 1),
                            )
                        o_sb = opool.tile([128, 512], F32, tag="o")
                        nc.vector.tensor_scalar_add(
                            out=o_sb[:], in0=ps[:],
                            scalar1=tvec_sb[:, ni:ni + 1],
                        )
                        # Stores go on the scalar HWDGE FIFO so they never
                        # head-of-line-block the at-loads on the sync FIFO.
                        nc.scalar.dma_start(
                            out=out[ni * 128:(ni + 1) * 128,
                                    c * MCH + h * 512:c * MCH + (h + 1) * 512],
                            in_=o_sb[:],
                        )

    nc.compile()
    return nc


def _get_nc(b_val: float):
    key = float(b_val)
    if key not in _NC_CACHE:
        _NC_CACHE[key] = _build(key)
    return _NC_CACHE[key]


def make_in_maps(target_spans, argument_spans, W, U, b):
    """Host-side layout prep: shard/transpose/cast the full inputs into the
    per-core input maps. Returns (in_maps, b_val)."""
    target_spans = np.asarray(target_spans, dtype=np.float32)
    argument_spans = np.asarray(argument_spans, dtype=np.float32)
    W = np.ascontiguousarray(np.asarray(W, dtype=np.float16))
    U = np.asarray(U, dtype=np.float32).reshape(2 * H, 1)
    b_val = float(np.asarray(b).reshape(-1)[0])

    tT = np.ascontiguousarray(target_spans.T.astype(np.float16))  # [H, N_TOT]
    aT = np.ascontiguousarray(argument_spans.T.astype(np.float16))  # [H, M_TOT]
    ut = np.ascontiguousarray(
        np.repeat(U[:H], 2, axis=1).astype(np.float16))  # [H, 2]
    ua = np.ascontiguousarray(U[H:])

    in_maps = [
        {
            "tT": np.ascontiguousarray(tT[:, i * NSH:(i + 1) * NSH]),
            "aT": aT,
            "W": W,
            "ut": ut,
            "ua": ua,
        }
        for i in range(N_CORES)
    ]
    return in_maps, b_val


def kernel(target_spans, argument_spans, W, U, b):
    in_maps, b_val = make_in_maps(target_spans, argument_spans, W, U, b)
    nc = _get_nc(b_val)
    res = run_bass_kernel_spmd(nc, in_maps, core_ids=list(range(N_CORES)))
    out = np.concatenate(
        [res.results[i]["out"] for i in range(N_CORES)], axis=0
    )
    return out.astype(np.float32, copy=False)
